# revision 3
# baseline (speedup 1.0000x reference)
"""MultiHeadGAT layer as a Trainium2 Bass kernel (8-core SPMD).

Strategy (for N=50000, E=1600000, F=256, HEADS=8, HD=32):
  - Host: permute nodes by in-degree; deal 128-node target tiles round-robin
    to 8 cores so compile-time slot capacities are shared by all cores.
  - Phase A (per core): compute a contiguous N/8-row shard of a bf16 node
    table [H(256) | sj(8) | pad] -> AllGather to the full [N, 320] table.
  - Phase B (per pair of target tiles): dma_gather neighbor rows in
    slot-major order -> grid [128, cap, 320] with target node == partition;
    softmax weights via per-partition broadcasts (segment-max subtraction
    dropped: softmax is shift-invariant and logits here are O(1)); scale
    gathered features by alpha on DVE (pair-duplicated bf16 alpha to hit the
    packed 2x DVE mode); reduce over slots with identity-weight PE matmuls
    accumulating in PSUM.  Then skip + ELU + LayerNorm + head mean + output
    matmul + ELU.
  - int16 gather-index limit (32767) handled by splitting each tile's slots
    into a "lo" grid (table rows < 32768) and a "hi" grid (rows >= 32768,
    gathered from an offset view of the table).  Padding slots point at row
    0 and are suppressed by a -1e30 additive mask before exp.
"""

import sys

sys.path.insert(0, "/opt/trn_rl_repo")

import numpy as np
import ml_dtypes

import concourse.bass as bass
import concourse.bacc as bacc
import concourse.mybir as mybir
import concourse.tile as tile
from concourse.library_config import mlp
from concourse.tile_rust import add_dep_helper

# ---------------------------------------------------------------- constants
F_IN = 256
HID = 256
HEADS = 8
HD = 32
SLOPE = 0.2
EPS = 1e-5
P = 128
SPLIT = 32768
TBL_W = 384  # bf16 table row: 256 H + 8 sj + pad  (768 B, %256 == 0)
MASK_BIG = 1.0e30
RB = 8  # slot-block size for the R buffer

N_NODES = 50000
N_EDGES = 1600000
NC = 8
G_TILES = 392  # 392*128 = 50176 >= 50000; 392 % 8 == 0
NO_GATHER = False  # debug: replace dma_gather with memset

F32 = mybir.dt.float32
BF16 = mybir.dt.bfloat16
I16 = mybir.dt.int16
AF = mybir.ActivationFunctionType
OP = mybir.AluOpType
AX = mybir.AxisListType


def _dims():
    lt = G_TILES // NC
    return dict(
        LT=lt,
        NPAD=G_TILES * P,
        SHARD=N_NODES // NC,
        n_pairs=(lt + 1) // 2,
    )


# ---------------------------------------------------------------- host prep
def _prepare(node_features, edge_index, W, bw, A, ba, gamma, beta, Wout, bout):
    d = _dims()
    LT, NPAD, SHARD, n_pairs = d["LT"], d["NPAD"], d["SHARD"], d["n_pairs"]

    x = np.asarray(node_features, np.float32)
    tgt = np.asarray(edge_index[0], np.int64)
    src = np.asarray(edge_index[1], np.int64)

    deg = np.bincount(tgt, minlength=N_NODES)
    perm = np.argsort(deg, kind="stable")
    rank = np.empty(N_NODES, np.int64)
    rank[perm] = np.arange(N_NODES)

    r_tgt = rank[tgt]
    r_src = rank[src]
    is_hi = r_src >= SPLIT

    order = np.lexsort((is_hi, r_tgt))
    rt_s = r_tgt[order]
    rs_s = r_src[order]
    hi_s = is_hi[order]

    deg_lo = np.bincount(rt_s[~hi_s], minlength=NPAD)
    deg_hi = np.bincount(rt_s[hi_s], minlength=NPAD)
    starts = np.zeros(NPAD + 1, np.int64)
    np.cumsum(np.bincount(rt_s, minlength=NPAD), out=starts[1:])

    # per-pair caps shared by all cores (same compiled program everywhere)
    dlo_t = deg_lo.reshape(G_TILES, P).max(axis=1)  # per global tile
    dhi_t = deg_hi.reshape(G_TILES, P).max(axis=1)
    cap_lo = np.zeros(n_pairs, np.int64)
    cap_hi = np.zeros(n_pairs, np.int64)
    for pi in range(n_pairs):
        lts = [t for t in (2 * pi, 2 * pi + 1) if t < LT]
        gs = [t * NC + c for t in lts for c in range(NC)]
        cap_lo[pi] = 2 * ((max(1, int(dlo_t[gs].max())) + 1) // 2)
        cap_hi[pi] = 2 * ((max(1, int(dhi_t[gs].max())) + 1) // 2)

    tiles_cap = np.zeros(LT, np.int64)  # cap per local tile
    for pi in range(n_pairs):
        for t in (2 * pi, 2 * pi + 1):
            if t < LT:
                tiles_cap[t] = cap_lo[pi] + cap_hi[pi]
    idx_cols = int(8 * tiles_cap.sum())
    mask_cols = int(tiles_cap.sum())

    x_pad = np.zeros((NPAD, F_IN), np.float32)
    x_pad[:N_NODES] = x[perm]

    in_maps = []
    for c in range(NC):
        idx_arr = np.zeros((P, idx_cols), np.int16)
        mask_arr = np.full((P, mask_cols), MASK_BIG, np.float32)
        icol = 0
        mcol = 0
        for pi in range(n_pairs):
            cl, ch = int(cap_lo[pi]), int(cap_hi[pi])
            for t in (2 * pi, 2 * pi + 1):
                if t >= LT:
                    continue
                g = t * NC + c
                ilo = np.zeros(P * cl, np.int16)
                ihi = np.zeros(P * ch, np.int16)
                for p in range(P):
                    r = g * P + p
                    if r >= N_NODES:
                        continue
                    s0 = starts[r]
                    dl = int(deg_lo[r])
                    dh = int(deg_hi[r])
                    if dl:
                        ilo[p : p + P * dl : P] = rs_s[s0 : s0 + dl]
                        mask_arr[p, mcol : mcol + dl] = 0.0
                    if dh:
                        ihi[p : p + P * dh : P] = rs_s[s0 + dl : s0 + dl + dh] - SPLIT
                        mask_arr[p, mcol + cl : mcol + cl + dh] = 0.0
                idx_arr[:, icol : icol + 8 * cl] = np.tile(
                    ilo.reshape(-1, 16).T, (8, 1)
                )
                idx_arr[:, icol + 8 * cl : icol + 8 * (cl + ch)] = np.tile(
                    ihi.reshape(-1, 16).T, (8, 1)
                )
                icol += 8 * (cl + ch)
                mcol += cl + ch

        g_idx = (np.arange(LT) * NC + c)[:, None] * P + np.arange(P)[None, :]
        x_tgt = x_pad[g_idx.reshape(-1)]
        x_tbl = np.zeros((LT * P, F_IN), np.float32)
        x_tbl[:SHARD] = x_pad[c * SHARD : (c + 1) * SHARD]
        in_maps.append(
            dict(
                x_tgt=np.ascontiguousarray(x_tgt),
                x_tbl=np.ascontiguousarray(x_tbl),
                idx=idx_arr,
                mask=mask_arr,
            )
        )

    # weight-space constants (host-side weight folding only)
    W = np.asarray(W, np.float32)
    bw_ = np.asarray(bw, np.float32)
    A_ = np.asarray(A, np.float32)
    ba_ = np.asarray(ba, np.float32)
    Wcat = np.zeros((F_IN, HID), np.float32)
    va = np.zeros((F_IN, 2 * HEADS), np.float32)
    c1 = np.zeros(HEADS, np.float32)
    c2 = np.zeros(HEADS, np.float32)
    for h in range(HEADS):
        Wcat[:, h * HD : (h + 1) * HD] = W[h]
        va[:, h] = W[h] @ A_[h, :HD]
        va[:, HEADS + h] = W[h] @ A_[h, HD:]
        c1[h] = float(bw_[h] @ A_[h, :HD] + ba_[h])
        c2[h] = float(bw_[h] @ A_[h, HD:])

    gamma = np.asarray(gamma, np.float32)
    beta = np.asarray(beta, np.float32)
    consts = dict(
        WCAT=np.ascontiguousarray(Wcat.reshape(2, 128, HID)),
        VA=np.ascontiguousarray(va.reshape(2, 128, 2 * HEADS)),
        BWEXP=np.tile(bw_.reshape(1, -1), (P, 1)),
        C1E=np.tile(c1.reshape(1, -1), (P, 1)),
        C2E=np.tile(c2.reshape(1, -1), (P, 1)),
        WOUT=np.ascontiguousarray(np.asarray(Wout, np.float32)),
        BOUTEXP=np.tile(np.asarray(bout, np.float32).reshape(1, -1), (P, 1)),
        GEXP=np.tile(gamma.reshape(1, -1), (P, 1)),
        MBETA=np.tile(beta.mean(axis=0).reshape(1, -1), (P, 1)),
        IDENT_BF=np.eye(P, dtype=ml_dtypes.bfloat16),
        IDENT_F32=np.eye(P, dtype=np.float32),
    )
    meta = dict(
        cap_lo=cap_lo,
        cap_hi=cap_hi,
        idx_cols=idx_cols,
        mask_cols=mask_cols,
        perm=perm,
        gamma_trivial=bool(np.allclose(gamma, 1.0) and np.allclose(beta, 0.0)),
    )
    return meta, in_maps, consts


# ------------------------------------------------------------- device build
def _build(meta, consts):
    d = _dims()
    LT, SHARD, n_pairs = d["LT"], d["SHARD"], d["n_pairs"]
    cap_lo, cap_hi = meta["cap_lo"], meta["cap_hi"]
    gamma_trivial = meta["gamma_trivial"]

    nc = bacc.Bacc(None, num_devices=NC)

    x_tgt = nc.dram_tensor("x_tgt", [LT * P, F_IN], F32, kind="ExternalInput")
    x_tbl = nc.dram_tensor("x_tbl", [LT * P, F_IN], F32, kind="ExternalInput")
    idx_d = nc.dram_tensor("idx", [P, meta["idx_cols"]], I16, kind="ExternalInput")
    mask_d = nc.dram_tensor("mask", [P, meta["mask_cols"]], F32, kind="ExternalInput")
    y_out = nc.dram_tensor("y", [LT * P, HID], F32, kind="ExternalOutput")

    tbl_shard = nc.dram_tensor("tbl_shard", [SHARD, TBL_W], BF16)
    tbl_full = nc.dram_tensor("tbl_full", [N_NODES, TBL_W], BF16, addr_space="Shared")

    cWCAT = nc.inline_tensor(consts["WCAT"], "cWCAT")
    cVA = nc.inline_tensor(consts["VA"], "cVA")
    cBWEXP = nc.inline_tensor(consts["BWEXP"], "cBWEXP")
    cC1E = nc.inline_tensor(consts["C1E"], "cC1E")
    cC2E = nc.inline_tensor(consts["C2E"], "cC2E")
    cWOUT = nc.inline_tensor(consts["WOUT"], "cWOUT")
    cBOUT = nc.inline_tensor(consts["BOUTEXP"], "cBOUT")
    cIDB = nc.inline_tensor(np.asarray(consts["IDENT_BF"]), "cIDB")
    cIDF = nc.inline_tensor(consts["IDENT_F32"], "cIDF")
    if not gamma_trivial:
        cGEXP = nc.inline_tensor(consts["GEXP"], "cGEXP")
        cMBETA = nc.inline_tensor(consts["MBETA"], "cMBETA")

    nidx_reg = {}
    for v in sorted({P * int(c) for c in cap_lo} | {P * int(c) for c in cap_hi}):
        r = nc.alloc_register(mybir.EngineType.Pool, f"nidx{v}")
        nc.gpsimd.reg_mov(r, v)
        nidx_reg[v] = r

    with tile.TileContext(nc) as tc:
        with (
            tc.tile_pool(name="const", bufs=1) as cpool,
            tc.tile_pool(name="small", bufs=2) as spool,
            tc.tile_pool(name="grid", bufs=2) as gpool,
            tc.tile_pool(name="rbuf", bufs=2) as rpool,
            tc.tile_pool(name="post", bufs=2) as ppool,
            tc.tile_pool(name="psA", bufs=2, space="PSUM") as psA,
            tc.tile_pool(name="psB", bufs=2, space="PSUM") as psB,
            tc.tile_pool(name="psC", bufs=2, space="PSUM") as psC,
        ):
            WCAT = cpool.tile([P, 2, HID], F32)
            VA = cpool.tile([P, 2, 2 * HEADS], F32)
            BWEXP = cpool.tile([P, HID], F32)
            C1E = cpool.tile([P, HEADS], F32)
            C2E = cpool.tile([P, HEADS], F32)
            WOUT = cpool.tile([HD, HID], F32)
            BOUT = cpool.tile([P, HID], F32)
            IDB = cpool.tile([P, P], BF16)
            IDF = cpool.tile([P, P], F32)
            EPSC = cpool.tile([P, 1], F32)
            nc.gpsimd.memset(EPSC[:], EPS)
            nc.sync.dma_start(WCAT[:], cWCAT[:].rearrange("k p n -> p k n"))
            nc.sync.dma_start(VA[:], cVA[:].rearrange("k p n -> p k n"))
            nc.sync.dma_start(BWEXP[:], cBWEXP[:])
            nc.sync.dma_start(C1E[:], cC1E[:])
            nc.sync.dma_start(C2E[:], cC2E[:])
            nc.sync.dma_start(WOUT[:], cWOUT[:])
            nc.sync.dma_start(BOUT[:], cBOUT[:])
            nc.sync.dma_start(IDB[:], cIDB[:])
            nc.sync.dma_start(IDF[:], cIDF[:])
            if not gamma_trivial:
                GEXP = cpool.tile([P, HID], F32)
                MBETA = cpool.tile([P, HD], F32)
                nc.sync.dma_start(GEXP[:], cGEXP[:])
                nc.sync.dma_start(MBETA[:], cMBETA[:])

            def hx_compute(x_dram, t, rows):
                """x tile -> (psum [128, 256+16] = [H | si | sj])."""
                xt = spool.tile([P, F_IN], F32, tag="xt")
                nc.sync.dma_start(xt[:rows], x_dram[t * P : t * P + rows, :])
                xT = spool.tile([P, 2, P], F32, tag="xT")
                for k in range(2):
                    pt = psC.tile([P, P], F32, tag="ps_tr")
                    nc.tensor.transpose(pt[:], xt[:, k * P : (k + 1) * P], IDF[:])
                    nc.scalar.copy(xT[:, k, :], pt[:])
                ph = psA.tile([P, F_IN + 2 * HEADS], F32, tag="ps_hx")
                for k in range(2):
                    nc.tensor.matmul(
                        ph[:, 0:HID],
                        xT[:, k, :],
                        WCAT[:, k, :],
                        start=(k == 0),
                        stop=(k == 1),
                    )
                for k in range(2):
                    nc.tensor.matmul(
                        ph[:, HID:],
                        xT[:, k, :],
                        VA[:, k, :],
                        start=(k == 0),
                        stop=(k == 1),
                    )
                return ph

            # ================= Phase A: table shard + AllGather ===========
            full_tiles, rem = divmod(SHARD, P)
            for t in range(full_tiles + (1 if rem else 0)):
                rows = P if t < full_tiles else rem
                ph = hx_compute(x_tbl, t, rows)
                tblt = spool.tile([P, TBL_W], BF16, tag="tblt")
                nc.vector.tensor_tensor(
                    out=tblt[:, 0:HID], in0=ph[:, 0:HID], in1=BWEXP[:], op=OP.add
                )
                # fill the whole row tail with copies of sj (keeps DRAM pad
                # initialized without a separate memset)
                nrep = (TBL_W - HID) // HEADS
                nc.vector.tensor_tensor(
                    out=tblt[:, HID:].rearrange("p (r h) -> p r h", h=HEADS),
                    in0=ph[:, HID + HEADS :]
                    .unsqueeze(1)
                    .to_broadcast([P, nrep, HEADS]),
                    in1=C2E[:].unsqueeze(1).to_broadcast([P, nrep, HEADS]),
                    op=OP.add,
                )
                nc.sync.dma_start(
                    tbl_shard[t * P : t * P + rows, :], tblt[:rows, :]
                )

            if NC > 1:
                nc.gpsimd.collective_compute(
                    "AllGather",
                    OP.bypass,
                    replica_groups=[list(range(NC))],
                    ins=[tbl_shard[:].opt()],
                    outs=[tbl_full[:].opt()],
                )
            else:
                nc.sync.dma_start(tbl_full[:, :], tbl_shard[:, :])

            # ================= Phase B ====================================
            icol = 0
            mcol = 0
            for pi in range(n_pairs):
                cl, ch = int(cap_lo[pi]), int(cap_hi[pi])
                cap = cl + ch
                tiles = [t for t in (2 * pi, 2 * pi + 1) if t < LT]
                ntl = len(tiles)

                hp = ppool.tile([P, 2, F_IN], F32, tag="hp")
                sibp = spool.tile([P, 2, HEADS], F32, tag="sibp")
                grids = []
                ex2s = []
                for ti, t in enumerate(tiles):
                    # h + sib for this tile
                    ph = hx_compute(x_tgt, t, P)
                    nc.vector.tensor_tensor(
                        out=hp[:, ti, :], in0=ph[:, 0:HID], in1=BWEXP[:], op=OP.add
                    )
                    nc.vector.tensor_tensor(
                        out=sibp[:, ti, :],
                        in0=ph[:, HID : HID + HEADS],
                        in1=C1E[:],
                        op=OP.add,
                    )

                    # gather
                    idxt = spool.tile([P, 8 * cap], I16, tag="idxt")
                    nc.sync.dma_start(idxt[:], idx_d[:, icol : icol + 8 * cap])
                    icol += 8 * cap
                    maskt = spool.tile([P, cap], F32, tag="maskt")
                    nc.sync.dma_start(maskt[:], mask_d[:, mcol : mcol + cap])
                    mcol += cap

                    grid = gpool.tile([P, cap, TBL_W], BF16, tag="grid")
                    if NO_GATHER:
                        nc.gpsimd.memset(grid[:], 0)
                    else:
                        nc.gpsimd.dma_gather(
                            grid[:, 0:cl, :],
                            tbl_full[:],
                            idxt[:, 0 : 8 * cl],
                            P * cl,
                            nidx_reg[P * cl],
                            TBL_W,
                            single_packet=False,
                        )
                        nc.gpsimd.dma_gather(
                            grid[:, cl:cap, :],
                            tbl_full[SPLIT:, :],
                            idxt[:, 8 * cl :],
                            P * ch,
                            nidx_reg[P * ch],
                            TBL_W,
                            single_packet=False,
                        )


                    # e = sj + sib - mask ; lrelu ; exp
                    eg = spool.tile([P, HEADS, cap], F32, tag="eg")
                    nc.vector.tensor_tensor(
                        out=eg[:],
                        in0=grid[:, :, HID : HID + HEADS].transpose([0, 2, 1]),
                        in1=sibp[:, ti, :].unsqueeze(2).to_broadcast([P, HEADS, cap]),
                        op=OP.add,
                    )
                    nc.vector.tensor_tensor(
                        out=eg[:],
                        in0=eg[:],
                        in1=maskt[:].unsqueeze(1).to_broadcast([P, HEADS, cap]),
                        op=OP.subtract,
                    )
                    nc.vector.scalar_tensor_tensor(
                        out=eg[:], in0=eg[:], scalar=SLOPE, in1=eg[:],
                        op0=OP.mult, op1=OP.max,
                    )
                    exg = spool.tile([P, HEADS, cap], F32, tag="exg")
                    nc.scalar.activation(exg[:], eg[:], AF.Exp)

                    den = spool.tile([P, HEADS], F32, tag="den")
                    nc.vector.tensor_reduce(den[:], exg[:], axis=AX.X, op=OP.add)
                    nc.vector.tensor_scalar_max(den[:], den[:], 1e-30)
                    rden = spool.tile([P, HEADS], F32, tag="rden")
                    nc.vector.reciprocal(rden[:], den[:])

                    # alpha pairs (bf16): ex2[c, h, 2] = exg[h, c] * rden[h]
                    ex2 = spool.tile([P, cap, HEADS, 2], BF16, tag="ex2")
                    nc.vector.tensor_tensor(
                        out=ex2[:],
                        in0=exg[:]
                        .transpose([0, 2, 1])
                        .unsqueeze(3)
                        .to_broadcast([P, cap, HEADS, 2]),
                        in1=rden[:]
                        .unsqueeze(1)
                        .unsqueeze(3)
                        .to_broadcast([P, cap, HEADS, 2]),
                        op=OP.mult,
                    )
                    grids.append(grid)
                    ex2s.append(ex2)

                # R blocks: R[:, j, ti, :] = alpha * Hsrc ; PE accumulates
                pagg = psB.tile([P, 2, HID], F32, tag="ps_big")
                aggv = pagg[:, 0:ntl, :].rearrange("p t n -> p (t n)")
                nblk = (cap + RB - 1) // RB
                ci = 0
                for b in range(nblk):
                    nb = min(RB, cap - b * RB)
                    Rc = rpool.tile([P, RB, 2, HID], BF16, tag="R")
                    for ti in range(ntl):
                        nc.vector.tensor_tensor(
                            out=Rc[:, 0:nb, ti, :].rearrange(
                                "p c (h f two) -> p c h f two", h=HEADS, two=2
                            ),
                            in0=grids[ti][:, b * RB : b * RB + nb, 0:HID].rearrange(
                                "p c (h f two) -> p c h f two", h=HEADS, two=2
                            ),
                            in1=ex2s[ti][:, b * RB : b * RB + nb, :, :]
                            .unsqueeze(3)
                            .to_broadcast([P, nb, HEADS, HD // 2, 2]),
                            op=OP.mult,
                        )
                    for j in range(nb):
                        nc.tensor.matmul(
                            aggv,
                            IDB[:],
                            Rc[:, j, 0:ntl, :].rearrange("p t n -> p (t n)"),
                            start=(ci == 0),
                            stop=(ci == cap - 1),
                        )
                        ci += 1

                # ---- post: skip + ELU + LN + head mean + out matmul + ELU
                ob = ppool.tile([P, 2, HID], F32, tag="ob")
                nc.vector.tensor_tensor(
                    out=ob[:, 0:ntl, :],
                    in0=pagg[:, 0:ntl, :],
                    in1=hp[:, 0:ntl, :],
                    op=OP.add,
                )
                t1 = ppool.tile([P, 2, HID], F32, tag="t1")
                nc.scalar.activation(t1[:, 0:ntl, :], ob[:, 0:ntl, :], AF.Relu, scale=-1.0)
                nc.scalar.activation(
                    t1[:, 0:ntl, :], t1[:, 0:ntl, :], AF.Exp, scale=-1.0
                )
                elu = ppool.tile([P, 2, HID], F32, tag="elu")
                nc.vector.scalar_tensor_tensor(
                    out=elu[:, 0:ntl, :],
                    in0=t1[:, 0:ntl, :],
                    scalar=-1.0,
                    in1=ob[:, 0:ntl, :],
                    op0=OP.add,
                    op1=OP.max,
                )

                nh = ntl * HEADS
                st = ppool.tile([P, 8, 2 * HEADS], F32, tag="st")  # LN scratch
                r1, r2, mu, mu2, var, sd, rr, tmp = (st[:, i, :] for i in range(8))
                nc.vector.tensor_reduce(
                    r1[:, 0:nh],
                    elu[:, 0:ntl, :].rearrange("p t (h f) -> p (t h) f", f=HD),
                    axis=AX.X,
                    op=OP.add,
                )
                sq = ppool.tile([P, 2, HID], F32, tag="t1")
                nc.scalar.activation(sq[:, 0:ntl, :], elu[:, 0:ntl, :], AF.Square)
                nc.vector.tensor_reduce(
                    r2[:, 0:nh],
                    sq[:, 0:ntl, :].rearrange("p t (h f) -> p (t h) f", f=HD),
                    axis=AX.X,
                    op=OP.add,
                )
                nc.vector.tensor_scalar_mul(mu[:, 0:nh], r1[:, 0:nh], 1.0 / HD)
                nc.scalar.activation(mu2[:, 0:nh], mu[:, 0:nh], AF.Square)
                nc.vector.scalar_tensor_tensor(
                    out=var[:, 0:nh],
                    in0=r2[:, 0:nh],
                    scalar=1.0 / HD,
                    in1=mu2[:, 0:nh],
                    op0=OP.mult,
                    op1=OP.subtract,
                )
                nc.scalar.activation(sd[:, 0:nh], var[:, 0:nh], AF.Sqrt, bias=EPSC[:])
                nc.vector.tensor_scalar_mul(sd[:, 0:nh], sd[:, 0:nh], float(HEADS))
                nc.vector.reciprocal(rr[:, 0:nh], sd[:, 0:nh])  # rstd / 8
                nc.vector.tensor_tensor(
                    out=tmp[:, 0:nh], in0=mu[:, 0:nh], in1=rr[:, 0:nh], op=OP.mult
                )
                ct = spool.tile([P, 2], F32, tag="ct")
                nc.vector.tensor_reduce(
                    ct[:, 0:ntl],
                    tmp[:, 0:nh].rearrange("p (t h) -> p t h", h=HEADS),
                    axis=AX.X,
                    op=OP.add,
                )

                xw = ppool.tile([P, 2, HID], F32, tag="ob")
                rr_b = (
                    rr[:, 0:nh]
                    .rearrange("p (t h) -> p t h", h=HEADS)
                    .unsqueeze(3)
                    .to_broadcast([P, ntl, HEADS, HD])
                )
                nc.vector.tensor_tensor(
                    out=xw[:, 0:ntl, :].rearrange("p t (h f) -> p t h f", h=HEADS),
                    in0=elu[:, 0:ntl, :].rearrange("p t (h f) -> p t h f", h=HEADS),
                    in1=rr_b,
                    op=OP.mult,
                )
                if not gamma_trivial:
                    nc.vector.tensor_tensor(
                        out=xw[:, 0:ntl, :],
                        in0=xw[:, 0:ntl, :],
                        in1=GEXP[:].unsqueeze(1).to_broadcast([P, ntl, HID]),
                        op=OP.mult,
                    )
                mh = ppool.tile([P, 2, HD], F32, tag="mh")
                nc.vector.tensor_reduce(
                    mh[:, 0:ntl, :],
                    xw[:, 0:ntl, :].rearrange("p t (h f) -> p t f h", f=HD),
                    axis=AX.X,
                    op=OP.add,
                )
                mhc = ppool.tile([P, 2, HD], F32, tag="mhc")
                nc.vector.tensor_tensor(
                    out=mhc[:, 0:ntl, :],
                    in0=mh[:, 0:ntl, :],
                    in1=ct[:, 0:ntl].unsqueeze(2).to_broadcast([P, ntl, HD]),
                    op=OP.subtract,
                )
                if not gamma_trivial:
                    nc.vector.tensor_tensor(
                        out=mhc[:, 0:ntl, :],
                        in0=mhc[:, 0:ntl, :],
                        in1=MBETA[:].unsqueeze(1).to_broadcast([P, ntl, HD]),
                        op=OP.add,
                    )

                py = psB.tile([P, 2, HID], F32, tag="ps_big")
                for ti in range(ntl):
                    pt = psC.tile([P, P], F32, tag="ps_tr")
                    nc.tensor.transpose(pt[0:HD, :], mhc[:, ti, :], IDF[:])
                    mT = spool.tile([HD, P], F32, tag="mT")
                    nc.scalar.copy(mT[:], pt[0:HD, :])
                    nc.tensor.matmul(
                        py[:, ti, :], mT[:], WOUT[:], start=True, stop=True
                    )
                yb = ppool.tile([P, 2, HID], F32, tag="yb")
                nc.vector.tensor_tensor(
                    out=yb[:, 0:ntl, :],
                    in0=py[:, 0:ntl, :],
                    in1=BOUT[:].unsqueeze(1).to_broadcast([P, ntl, HID]),
                    op=OP.add,
                )
                t2 = ppool.tile([P, 2, HID], F32, tag="t1")
                nc.scalar.activation(
                    t2[:, 0:ntl, :], yb[:, 0:ntl, :], AF.Relu, scale=-1.0
                )
                nc.scalar.activation(
                    t2[:, 0:ntl, :], t2[:, 0:ntl, :], AF.Exp, scale=-1.0
                )
                ysb = ppool.tile([P, 2, HID], F32, tag="ob")
                nc.vector.scalar_tensor_tensor(
                    out=ysb[:, 0:ntl, :],
                    in0=t2[:, 0:ntl, :],
                    scalar=-1.0,
                    in1=yb[:, 0:ntl, :],
                    op0=OP.add,
                    op1=OP.max,
                )
                for ti, t in enumerate(tiles):
                    nc.sync.dma_start(
                        y_out[t * P : (t + 1) * P, :], ysb[:, ti, :]
                    )

    nc.compile()
    return nc


# ------------------------------------------------------------------ driver
_CACHE = {}


def kernel(**inputs):
    meta, in_maps, consts = _prepare(**inputs)
    key = (
        tuple(meta["cap_lo"].tolist()),
        tuple(meta["cap_hi"].tolist()),
        meta["gamma_trivial"],
    )
    if key not in _CACHE:
        _CACHE[key] = _build(meta, consts)
    nc = _CACHE[key]

    from concourse.bass_utils import run_bass_kernel_spmd

    global LAST_NC, LAST_INMAPS
    LAST_NC = nc
    LAST_INMAPS = in_maps

    res = run_bass_kernel_spmd(nc, in_maps, core_ids=list(range(NC)))
    global LAST_RESULT
    LAST_RESULT = res
    outs = res.results

    d = _dims()
    LT, NPAD = d["LT"], d["NPAD"]
    y_all = np.zeros((NPAD, HID), np.float32)
    for c in range(NC):
        g_idx = (np.arange(LT) * NC + c)[:, None] * P + np.arange(P)[None, :]
        y_all[g_idx.reshape(-1)] = outs[c]["y"]
    y = np.zeros((N_NODES, HID), np.float32)
    y[meta["perm"]] = y_all[:N_NODES]
    return y



# revision 20
# speedup vs baseline: 1.0126x; 1.0126x over previous
"""MultiHeadGAT layer as a Trainium2 Bass kernel (8-core SPMD), v3.

Design (N=50000, E=1.6M, F=256, HEADS=8, HD=32):
  - Host: permute nodes by in-degree; deal 128-node target tiles round-robin
    to 8 cores; per-tile slot cap (shared by all cores) + 1 sentinel slot.
    Each core gets its OWN node->table-row permutation (its targets first,
    in tile order) carried by its xT input + gather indices, so all DMA
    addresses are compile-time constants (SPMD-safe).
  - Phase A (every core): full node table computed locally (no AllGather).
    H = x@W + bw via PE from a transposed bf16 x; attention-logit linear
    terms si/sj (constants folded) computed in the same matmul; biases via a
    ones-row matmul + a DVE add.  Table row: 384 x bf16 (768 B) =
    [256 H | 16 sj-dup-pairs | 16 si-dup-pairs | 96 pad].  One extra pad row
    holds sj = -240 so padding slots vanish under exp() (no mask).
  - Phase B (per target tile): ONE dma_gather with *signed* int16 indices
    against a base-offset table view (base row 32768) covering all 50k rows
    (no lo/hi split); the gather's LAST index is the sentinel (non-negative)
    to dodge the trailing-negative DGE quirk.  Logits from gathered sj-dup;
    R = exp * H on DVE (bf16 2x packed); slot reduction via identity-weight
    PE matmuls in PSUM; normalize by 1/den post-reduction; skip + ELU +
    per-head LayerNorm + head-mean + output matmul (head-mean, gamma, beta,
    bout and the -mu*rstd LN correction folded into PE weights) + ELU;
    y stored bf16.
"""

import os
import sys

sys.path.insert(0, "/opt/trn_rl_repo")

import numpy as np
import ml_dtypes

import concourse.bass as bass
import concourse.bacc as bacc
import concourse.mybir as mybir
import concourse.tile as tile

# ---------------------------------------------------------------- constants
F_IN = 256
HID = 256
HEADS = 8
HD = 32
SLOPE = 0.2
EPS = 1e-5
P = 128
BASE = 32768

N_NODES = 50000
NC = 8
G_TILES = 392  # 392*128 = 50176 >= 50000; 392 % 8 == 0
NPAD = G_TILES * P
PADROW = NPAD
NR = NPAD + 16
TBW = 384  # bf16 elems per table row (768 B)
RB = 8
ABATCH = 4
NO_GATHER = False

F32 = mybir.dt.float32
BF16 = mybir.dt.bfloat16
I16 = mybir.dt.int16
AF = mybir.ActivationFunctionType
OP = mybir.AluOpType
AX = mybir.AxisListType

LT = G_TILES // NC


# ---------------------------------------------------------------- host prep
def _prepare(node_features, edge_index, W, bw, A, ba, gamma, beta, Wout, bout):
    x = np.asarray(node_features, np.float32)
    tgt = np.asarray(edge_index[0], np.int64)
    src = np.asarray(edge_index[1], np.int64)

    deg = np.bincount(tgt, minlength=N_NODES)
    perm = np.argsort(deg, kind="stable")
    rank = np.empty(N_NODES, np.int64)
    rank[perm] = np.arange(N_NODES)

    r_tgt = rank[tgt]
    r_src = rank[src]
    order = np.argsort(r_tgt, kind="stable")
    rt_s = r_tgt[order]
    rs_s = r_src[order]

    degp = np.bincount(rt_s, minlength=NPAD)
    starts = np.zeros(NPAD + 1, np.int64)
    np.cumsum(degp, out=starts[1:])
    dt_tile = degp.reshape(G_TILES, P).max(axis=1)

    # per-tile cap over the 8-core tile group, rounded to even (register
    # economy), +1 sentinel slot appended at use sites
    tile_cap = np.zeros(LT, np.int64)
    for t in range(LT):
        gs = [t * NC + c for c in range(NC)]
        tile_cap[t] = 2 * ((max(1, int(dt_tile[gs].max())) + 1) // 2)
    icols = int(8 * (tile_cap + 1).sum())

    x_pad = np.zeros((NPAD, F_IN), np.float32)
    x_pad[:N_NODES] = x[perm]

    ar = np.arange(NPAD)
    in_maps = []
    for c in range(NC):
        own = ((ar[: LT * P] // P) * NC + c) * P + (ar[: LT * P] % P)
        rest_mask = np.ones(NPAD, bool)
        rest_mask[own] = False
        pi_c = np.concatenate([own, ar[rest_mask]])
        rowc = np.empty(NPAD, np.int64)
        rowc[pi_c] = ar

        row_src = rowc[rs_s]
        idx_arr = np.zeros((P, icols), np.int16)
        icol = 0
        for t in range(LT):
            g = t * NC + c
            cap = int(tile_cap[t]) + 1
            fl = np.full(P * cap, PADROW, np.int64)
            for p in range(P):
                r = g * P + p
                dd = int(degp[r])
                if dd:
                    s0 = starts[r]
                    fl[p : p + P * dd : P] = row_src[s0 : s0 + dd]
            i16 = (fl - BASE).astype(np.int16)
            idx_arr[:, icol : icol + 8 * cap] = np.tile(
                i16.reshape(-1, 16).T, (8, 1)
            )
            icol += 8 * cap

        xT = x_pad[pi_c].T.astype(ml_dtypes.bfloat16)
        in_maps.append(
            dict(idx=idx_arr, xT=np.ascontiguousarray(xT.reshape(2, 128, NPAD)))
        )

    # ---- weight-space folding (host, f64)
    W = np.asarray(W, np.float64)
    bw_ = np.asarray(bw, np.float64)
    A_ = np.asarray(A, np.float64)
    ba_ = np.asarray(ba, np.float64)
    gamma = np.asarray(gamma, np.float64)
    beta = np.asarray(beta, np.float64)
    Wout_ = np.asarray(Wout, np.float64)
    bout_ = np.asarray(bout, np.float64)

    Wcat = np.zeros((F_IN, HID))
    va1 = np.zeros((F_IN, HEADS))
    va2 = np.zeros((F_IN, HEADS))
    c1 = np.zeros(HEADS)
    c2 = np.zeros(HEADS)
    for h in range(HEADS):
        Wcat[:, h * HD : (h + 1) * HD] = W[h]
        va1[:, h] = W[h] @ A_[h, :HD]
        va2[:, h] = W[h] @ A_[h, HD:]
        c1[h] = bw_[h] @ A_[h, :HD] + ba_[h]
        c2[h] = bw_[h] @ A_[h, HD:]

    # ph columns: [0:256) H(no bias) | [256:272) sj dup | [272:288) si dup
    WCATA = np.zeros((2, 128, 288))
    BIASROW = np.zeros((1, 288))
    for k in range(2):
        WCATA[k, :, 0:256] = Wcat[k * 128 : (k + 1) * 128, :]
        for h in range(HEADS):
            for r in range(2):
                WCATA[k, :, 256 + 2 * h + r] = va2[k * 128 : (k + 1) * 128, h]
                WCATA[k, :, 272 + 2 * h + r] = va1[k * 128 : (k + 1) * 128, h]
    for h in range(HEADS):
        for r in range(2):
            BIASROW[0, 256 + 2 * h + r] = c2[h]
            BIASROW[0, 272 + 2 * h + r] = c1[h]

    WTILE = np.zeros((2, 128, HID))
    for f in range(F_IN):
        h, j = f // HD, f % HD
        WTILE[f // 128, f % 128, :] = gamma[h, j] * Wout_[j, :] / HEADS
    WEXTRA = np.zeros((9, HID))
    for h in range(HEADS):
        WEXTRA[h] = -(gamma[h] @ Wout_) / (HEADS * HD)
    WEXTRA[8] = bout_ + beta.mean(axis=0) @ Wout_

    padrow = np.zeros(TBW, ml_dtypes.bfloat16)
    padrow[256:272] = -240.0

    consts = dict(
        WCATA=WCATA.astype(ml_dtypes.bfloat16),
        BIASROW=BIASROW.astype(ml_dtypes.bfloat16),
        BWEXP=np.tile(bw_.reshape(1, -1), (P, 1)).astype(ml_dtypes.bfloat16),
        WTILE=WTILE.astype(ml_dtypes.bfloat16),
        WEXTRA=WEXTRA.astype(ml_dtypes.bfloat16),
        PADROW=padrow.reshape(1, TBW),
        ONESB=np.ones((1, 128), ml_dtypes.bfloat16),
        IDB=np.eye(P, dtype=ml_dtypes.bfloat16),
        IDF=np.eye(P, dtype=np.float32),
    )
    meta = dict(tile_cap=tile_cap, icols=icols, perm=perm)
    return meta, in_maps, consts


# ------------------------------------------------------------- device build
def _build(meta, consts):
    tile_cap, icols = meta["tile_cap"], meta["icols"]

    nc = bacc.Bacc(None, num_devices=NC)

    xT_d = nc.dram_tensor("xT", [2, 128, NPAD], BF16, kind="ExternalInput")
    idx_d = nc.dram_tensor("idx", [P, icols], I16, kind="ExternalInput")
    y_d = nc.dram_tensor("y", [LT * P, HID], BF16, kind="ExternalOutput")
    tbl_d = nc.dram_tensor("tbl", [NR, TBW], BF16)
    debug = os.environ.get("K2_DEBUG", "") == "1"
    if debug:
        dbg_tbl = nc.dram_tensor(
            "dbg_tbl", [NR, 2 * TBW], mybir.dt.uint8, kind="ExternalOutput"
        )

    cWCATA = nc.inline_tensor(np.asarray(consts["WCATA"]), "cWCATA")
    cBIASROW = nc.inline_tensor(np.asarray(consts["BIASROW"]), "cBIASROW")
    cBWEXP = nc.inline_tensor(np.asarray(consts["BWEXP"]), "cBWEXP")
    cWTILE = nc.inline_tensor(np.asarray(consts["WTILE"]), "cWTILE")
    cWEXTRA = nc.inline_tensor(np.asarray(consts["WEXTRA"]), "cWEXTRA")
    cPADROW = nc.inline_tensor(np.asarray(consts["PADROW"]), "cPADROW")
    cONESB = nc.inline_tensor(np.asarray(consts["ONESB"]), "cONESB")
    cIDB = nc.inline_tensor(np.asarray(consts["IDB"]), "cIDB")
    cIDF = nc.inline_tensor(np.asarray(consts["IDF"]), "cIDF")

    nidx_reg = {}
    for v in sorted({P * (int(c) + 1) for c in tile_cap}):
        r = nc.alloc_register(mybir.EngineType.Pool, f"nidx{v}")
        nc.gpsimd.reg_mov(r, v)
        nidx_reg[v] = r

    with tile.TileContext(nc) as tc:
        with tc.tile_pool(name="const", bufs=1) as cpool:
            WCATA = cpool.tile([128, 2, 288], BF16)
            BIASROW = cpool.tile([1, 288], BF16)
            BWEXP = cpool.tile([P, 256], BF16)
            WTILE = cpool.tile([128, 2, HID], BF16)
            WEXTRA = cpool.tile([9, HID], BF16)
            PADT = cpool.tile([1, TBW], BF16)
            ONESB = cpool.tile([1, 128], BF16)
            IDB = cpool.tile([P, P], BF16)
            IDF = cpool.tile([P, P], F32)
            EPSC = cpool.tile([P, 1], F32)
            nc.gpsimd.memset(EPSC[:], EPS)
            nc.sync.dma_start(WCATA[:], cWCATA[:].rearrange("k p n -> p k n"))
            nc.sync.dma_start(BIASROW[:], cBIASROW[:])
            nc.sync.dma_start(BWEXP[:], cBWEXP[:])
            nc.sync.dma_start(WTILE[:], cWTILE[:].rearrange("k p n -> p k n"))
            nc.sync.dma_start(WEXTRA[:], cWEXTRA[:])
            nc.sync.dma_start(PADT[:], cPADROW[:])
            nc.sync.dma_start(ONESB[:], cONESB[:])
            nc.sync.dma_start(IDB[:], cIDB[:])
            nc.sync.dma_start(IDF[:], cIDF[:])

            # ================= Phase A: full node table ====================
            with (
                tc.tile_pool(name="xp", bufs=2) as xpool,
                tc.tile_pool(name="ap", bufs=2) as apool,
                tc.tile_pool(name="psA", bufs=2, space="PSUM") as psA,
            ):
                for b in range(G_TILES // ABATCH):
                    n0 = b * ABATCH * P
                    xb = xpool.tile([128, 2, ABATCH * P], BF16, tag="xb")
                    nc.sync.dma_start(
                        xb[:],
                        xT_d[:, :, n0 : n0 + ABATCH * P].rearrange(
                            "k p n -> p k n"
                        ),
                    )
                    ph = psA.tile([P, ABATCH, 512], F32, tag="phA")
                    for k in range(ABATCH):
                        nc.tensor.matmul(
                            ph[:, k, 0:288], xb[:, 0, k * P : (k + 1) * P],
                            WCATA[:, 0, :], start=True, stop=False,
                        )
                        nc.tensor.matmul(
                            ph[:, k, 0:256], xb[:, 1, k * P : (k + 1) * P],
                            WCATA[:, 1, 0:256], start=False, stop=True,
                        )
                        nc.tensor.matmul(
                            ph[:, k, 256:288], xb[:, 1, k * P : (k + 1) * P],
                            WCATA[:, 1, 256:288], start=False, stop=False,
                        )
                        nc.tensor.matmul(
                            ph[:, k, 256:288], ONESB[:], BIASROW[:, 256:288],
                            start=False, stop=True,
                        )
                    t8 = apool.tile([P, ABATCH, 288], BF16, tag="t8")
                    nc.vector.tensor_tensor(
                        out=t8[:, :, 0:256],
                        in0=ph[:, :, 0:256],
                        in1=BWEXP[:].unsqueeze(1).to_broadcast(
                            [P, ABATCH, 256]
                        ),
                        op=OP.add,
                    )
                    nc.scalar.copy(t8[:, :, 256:288], ph[:, :, 256:288])
                    nc.sync.dma_start(
                        tbl_d[n0 : n0 + ABATCH * P, 0:288].rearrange(
                            "(k p) w -> p k w", p=P
                        ),
                        t8[:],
                    )
                nc.sync.dma_start(tbl_d[PADROW : PADROW + 1, :], PADT[:])
                if debug:
                    nc.sync.dma_start(
                        dbg_tbl[:], tbl_d[:].bitcast(mybir.dt.uint8)
                    )

            # ================= Phase B =====================================
            with (
                tc.tile_pool(name="sp", bufs=2) as spool,
                tc.tile_pool(name="gp", bufs=2) as gpool,
                tc.tile_pool(name="rp", bufs=2) as rpool,
                tc.tile_pool(name="pp", bufs=2) as ppool,
                tc.tile_pool(name="psB", bufs=2, space="PSUM") as psB,
                tc.tile_pool(name="psC", bufs=2, space="PSUM") as psC,
            ):
                icol = 0
                for t in range(LT):
                    cap = int(tile_cap[t]) + 1
                    nblk = (cap + RB - 1) // RB

                    hblk = spool.tile([P, TBW], BF16, tag="hblk")
                    nc.sync.dma_start(hblk[:], tbl_d[t * P : (t + 1) * P, :])
                    idxt = spool.tile([P, 8 * cap], I16, tag="idxt")
                    nc.sync.dma_start(idxt[:], idx_d[:, icol : icol + 8 * cap])
                    icol += 8 * cap

                    grid = gpool.tile([P, cap, TBW], BF16, tag="grid")
                    if NO_GATHER:
                        nc.gpsimd.memset(grid[:], 0)
                    else:
                        nc.gpsimd.dma_gather(
                            grid[:],
                            tbl_d[BASE:, :],
                            idxt[:],
                            P * cap,
                            nidx_reg[P * cap],
                            TBW,
                            single_packet=False,
                        )

                    # logits -> exp (dup pairs), den
                    egd = spool.tile([P, cap, 16], BF16, tag="egd")
                    nc.vector.tensor_tensor(
                        out=egd[:],
                        in0=grid[:, :, 256:272],
                        in1=hblk[:, 272:288]
                        .unsqueeze(1)
                        .to_broadcast([P, cap, 16]),
                        op=OP.add,
                    )
                    nc.vector.scalar_tensor_tensor(
                        out=egd[:], in0=egd[:], scalar=SLOPE,
                        in1=egd[:], op0=OP.mult, op1=OP.max,
                    )
                    exd = spool.tile([P, cap, 16], BF16, tag="exd")
                    nc.scalar.activation(exd[:], egd[:], AF.Exp)
                    den = spool.tile([P, HEADS], F32, tag="den")
                    nc.vector.tensor_reduce(
                        den[:],
                        exd[:]
                        .rearrange("p c (h two) -> p h two c", two=2)[
                            :, :, 0, :
                        ],
                        axis=AX.X,
                        op=OP.add,
                    )
                    nc.vector.tensor_scalar_max(den[:], den[:], 1e-30)
                    rden = spool.tile([P, HEADS], F32, tag="rden")
                    nc.vector.reciprocal(rden[:], den[:])

                    # ---- R = exp * H ; PE reduces slots into PSUM
                    pagg = psB.tile([P, HID], F32, tag="ps_big")
                    ci = 0
                    for bb in range(nblk):
                        j0 = bb * RB
                        nb = min(RB, cap - j0)
                        Rc = rpool.tile([P, RB, HID], BF16, tag="R")
                        nc.vector.tensor_tensor(
                            out=Rc[:, 0:nb, :].rearrange(
                                "p c (h f two) -> p c h f two",
                                h=HEADS, two=2,
                            ),
                            in0=grid[:, j0 : j0 + nb, 0:256].rearrange(
                                "p c (h f two) -> p c h f two",
                                h=HEADS, two=2,
                            ),
                            in1=exd[:, j0 : j0 + nb, :]
                            .rearrange("p c (h two) -> p c h two", two=2)
                            .unsqueeze(3)
                            .to_broadcast([P, nb, HEADS, HD // 2, 2]),
                            op=OP.mult,
                        )
                        for j in range(nb):
                            nc.tensor.matmul(
                                pagg[:],
                                IDB[:],
                                Rc[:, j, :],
                                start=(ci == 0),
                                stop=(ci == cap - 1),
                            )
                            ci += 1

                    # ---- post: normalize, skip, ELU, LN, out matmul, ELU
                    ob = ppool.tile([P, HID], BF16, tag="ob")
                    nc.vector.tensor_tensor(
                        out=ob[:].rearrange("p (h f) -> p h f", h=HEADS),
                        in0=pagg[:].rearrange("p (h f) -> p h f", h=HEADS),
                        in1=rden[:].unsqueeze(2).to_broadcast([P, HEADS, HD]),
                        op=OP.mult,
                    )
                    nc.vector.tensor_tensor(
                        out=ob[:], in0=ob[:], in1=hblk[:, 0:256], op=OP.add
                    )
                    t1 = ppool.tile([P, HID], BF16, tag="t1")
                    nc.scalar.activation(t1[:], ob[:], AF.Relu, scale=-1.0)
                    nc.scalar.activation(t1[:], t1[:], AF.Exp, scale=-1.0)
                    elu = ppool.tile([P, HID], BF16, tag="elu")
                    nc.vector.scalar_tensor_tensor(
                        out=elu[:], in0=t1[:], scalar=-1.0, in1=ob[:],
                        op0=OP.add, op1=OP.max,
                    )

                    st = ppool.tile([P, 8, HEADS], F32, tag="st")
                    r1, r2, mu2, var, sd, rr, tmp, _ = (
                        st[:, i, :] for i in range(8)
                    )
                    nc.vector.tensor_reduce(
                        r1[:],
                        elu[:].rearrange("p (h f) -> p h f", f=HD),
                        axis=AX.X,
                        op=OP.add,
                    )
                    sq = ppool.tile([P, HID], BF16, tag="t1")
                    nc.scalar.activation(sq[:], elu[:], AF.Square)
                    nc.vector.tensor_reduce(
                        r2[:],
                        sq[:].rearrange("p (h f) -> p h f", f=HD),
                        axis=AX.X,
                        op=OP.add,
                    )
                    nc.scalar.activation(
                        mu2[:], r1[:], AF.Square, scale=1.0 / HD
                    )
                    nc.vector.scalar_tensor_tensor(
                        out=var[:], in0=r2[:], scalar=1.0 / HD, in1=mu2[:],
                        op0=OP.mult, op1=OP.subtract,
                    )
                    nc.scalar.activation(sd[:], var[:], AF.Sqrt, bias=EPSC[:])
                    nc.vector.reciprocal(rr[:], sd[:])
                    nc.vector.tensor_tensor(
                        out=tmp[:], in0=r1[:], in1=rr[:], op=OP.mult
                    )

                    xw = ppool.tile([P, HID], BF16, tag="xw")
                    nc.vector.tensor_tensor(
                        out=xw[:].rearrange("p (h f) -> p h f", h=HEADS),
                        in0=elu[:].rearrange("p (h f) -> p h f", h=HEADS),
                        in1=rr[:].unsqueeze(2).to_broadcast([P, HEADS, HD]),
                        op=OP.mult,
                    )

                    xwT = spool.tile([P, 2, P], BF16, tag="xwT")
                    for k in range(2):
                        pt = psC.tile([P, P], BF16, tag="ps_tr")
                        nc.tensor.transpose(
                            pt[:], xw[:, k * P : (k + 1) * P], IDB[:]
                        )
                        nc.scalar.copy(xwT[:, k, :], pt[:])
                    t9 = spool.tile([P, 9], F32, tag="t9")
                    nc.scalar.copy(t9[:, 0:8], tmp[:])
                    nc.scalar.activation(
                        t9[:, 8:9], t9[:, 0:1], AF.Copy, scale=0.0, bias=1.0
                    )
                    ptm = psC.tile([P, P], F32, tag="ps_trf")
                    nc.tensor.transpose(ptm[0:9, :], t9[:], IDF[:])
                    exT = spool.tile([9, P], BF16, tag="exT")
                    nc.scalar.copy(exT[:], ptm[0:9, :])

                    py = psB.tile([P, HID], F32, tag="ps_big")
                    nc.tensor.matmul(
                        py[:], xwT[:, 0, :], WTILE[:, 0, :],
                        start=True, stop=False,
                    )
                    nc.tensor.matmul(
                        py[:], xwT[:, 1, :], WTILE[:, 1, :],
                        start=False, stop=False,
                    )
                    nc.tensor.matmul(
                        py[:], exT[:], WEXTRA[:], start=False, stop=True
                    )
                    yb = ppool.tile([P, HID], BF16, tag="yb")
                    nc.scalar.copy(yb[:], py[:])
                    t2 = ppool.tile([P, HID], BF16, tag="t2")
                    nc.scalar.activation(t2[:], yb[:], AF.Relu, scale=-1.0)
                    nc.scalar.activation(t2[:], t2[:], AF.Exp, scale=-1.0)
                    ysb = ppool.tile([P, HID], BF16, tag="ysb")
                    nc.vector.scalar_tensor_tensor(
                        out=ysb[:], in0=t2[:], scalar=-1.0, in1=yb[:],
                        op0=OP.add, op1=OP.max,
                    )
                    nc.sync.dma_start(y_d[t * P : (t + 1) * P, :], ysb[:])

    nc.compile()
    return nc


# ------------------------------------------------------------------ driver
_CACHE = {}


def kernel(**inputs):
    meta, in_maps, consts = _prepare(**inputs)
    key = tuple(meta["tile_cap"].tolist())
    if key not in _CACHE:
        _CACHE[key] = _build(meta, consts)
    nc = _CACHE[key]

    from concourse.bass_utils import run_bass_kernel_spmd

    global LAST_NC, LAST_INMAPS
    LAST_NC = nc
    LAST_INMAPS = in_maps

    res = run_bass_kernel_spmd(nc, in_maps, core_ids=list(range(NC)))
    global LAST_RESULT
    LAST_RESULT = res
    outs = res.results

    y_all = np.zeros((NPAD, HID), np.float32)
    for c in range(NC):
        g_idx = (np.arange(LT) * NC + c)[:, None] * P + np.arange(P)[None, :]
        y_all[g_idx.reshape(-1)] = outs[c]["y"].astype(np.float32)
    y = np.zeros((N_NODES, HID), np.float32)
    y[meta["perm"]] = y_all[:N_NODES]
    return y


# revision 24
# speedup vs baseline: 1.0762x; 1.0628x over previous
"""MultiHeadGAT layer as a Trainium2 Bass kernel (8-core SPMD), v3.

Design (N=50000, E=1.6M, F=256, HEADS=8, HD=32):
  - Host: permute nodes by in-degree; deal 128-node target tiles round-robin
    to 8 cores; per-tile slot cap (shared by all cores) + 1 sentinel slot.
    Each core gets its OWN node->table-row permutation (its targets first,
    in tile order) carried by its xT input + gather indices, so all DMA
    addresses are compile-time constants (SPMD-safe).
  - Phase A (every core): full node table computed locally (no AllGather).
    H = x@W + bw via PE from a transposed bf16 x; attention-logit linear
    terms si/sj (constants folded) computed in the same matmul; biases via a
    ones-row matmul + a DVE add.  Table row: 384 x bf16 (768 B) =
    [256 H | 16 sj-dup-pairs | 16 si-dup-pairs | 96 pad].  One extra pad row
    holds sj = -240 so padding slots vanish under exp() (no mask).
  - Phase B (per target tile): ONE dma_gather with *signed* int16 indices
    against a base-offset table view (base row 32768) covering all 50k rows
    (no lo/hi split); the gather's LAST index is the sentinel (non-negative)
    to dodge the trailing-negative DGE quirk.  Logits from gathered sj-dup;
    R = exp * H on DVE (bf16 2x packed); slot reduction via identity-weight
    PE matmuls in PSUM; normalize by 1/den post-reduction; skip + ELU +
    per-head LayerNorm + head-mean + output matmul (head-mean, gamma, beta,
    bout and the -mu*rstd LN correction folded into PE weights) + ELU;
    y stored bf16.
"""

import os
import sys

sys.path.insert(0, "/opt/trn_rl_repo")

import numpy as np
import ml_dtypes

import concourse.bass as bass
import concourse.bacc as bacc
import concourse.mybir as mybir
import concourse.tile as tile

# ---------------------------------------------------------------- constants
F_IN = 256
HID = 256
HEADS = 8
HD = 32
SLOPE = 0.2
EPS = 1e-5
P = 128
BASE = 32768

N_NODES = 50000
NC = 8
G_TILES = 392  # 392*128 = 50176 >= 50000; 392 % 8 == 0
NPAD = G_TILES * P
PADROW = NPAD
NR = NPAD + 16
TBW = 384  # bf16 elems per table row (768 B)
RB = 8
ABATCH = 4
NO_GATHER = False

F32 = mybir.dt.float32
BF16 = mybir.dt.bfloat16
I16 = mybir.dt.int16
AF = mybir.ActivationFunctionType
OP = mybir.AluOpType
AX = mybir.AxisListType

LT = G_TILES // NC


# ---------------------------------------------------------------- host prep
def _prepare(node_features, edge_index, W, bw, A, ba, gamma, beta, Wout, bout):
    x = np.asarray(node_features, np.float32)
    tgt = np.asarray(edge_index[0], np.int64)
    src = np.asarray(edge_index[1], np.int64)

    deg = np.bincount(tgt, minlength=N_NODES)
    perm = np.argsort(deg, kind="stable")
    rank = np.empty(N_NODES, np.int64)
    rank[perm] = np.arange(N_NODES)

    r_tgt = rank[tgt]
    r_src = rank[src]
    order = np.argsort(r_tgt, kind="stable")
    rt_s = r_tgt[order]
    rs_s = r_src[order]

    degp = np.bincount(rt_s, minlength=NPAD)
    starts = np.zeros(NPAD + 1, np.int64)
    np.cumsum(degp, out=starts[1:])
    dt_tile = degp.reshape(G_TILES, P).max(axis=1)

    # per-tile cap over the 8-core tile group, rounded to even (register
    # economy), +1 sentinel slot appended at use sites
    tile_cap = np.zeros(LT, np.int64)
    for t in range(LT):
        gs = [t * NC + c for c in range(NC)]
        tile_cap[t] = 2 * ((max(1, int(dt_tile[gs].max())) + 1) // 2)
    icols = int(8 * (tile_cap + 1).sum())

    x_pad = np.zeros((NPAD, F_IN), np.float32)
    x_pad[:N_NODES] = x[perm]

    ar = np.arange(NPAD)
    in_maps = []
    for c in range(NC):
        own = ((ar[: LT * P] // P) * NC + c) * P + (ar[: LT * P] % P)
        rest_mask = np.ones(NPAD, bool)
        rest_mask[own] = False
        pi_c = np.concatenate([own, ar[rest_mask]])
        rowc = np.empty(NPAD, np.int64)
        rowc[pi_c] = ar

        row_src = rowc[rs_s]
        idx_arr = np.zeros((P, icols), np.int16)
        icol = 0
        for t in range(LT):
            g = t * NC + c
            cap = int(tile_cap[t]) + 1
            fl = np.full(P * cap, PADROW, np.int64)
            for p in range(P):
                r = g * P + p
                dd = int(degp[r])
                if dd:
                    s0 = starts[r]
                    fl[p : p + P * dd : P] = row_src[s0 : s0 + dd]
            i16 = (fl - BASE).astype(np.int16)
            idx_arr[:, icol : icol + 8 * cap] = np.tile(
                i16.reshape(-1, 16).T, (8, 1)
            )
            icol += 8 * cap

        xT = x_pad[pi_c].T.astype(ml_dtypes.bfloat16)
        in_maps.append(
            dict(idx=idx_arr, xT=np.ascontiguousarray(xT.reshape(2, 128, NPAD)))
        )

    # ---- weight-space folding (host, f64)
    W = np.asarray(W, np.float64)
    bw_ = np.asarray(bw, np.float64)
    A_ = np.asarray(A, np.float64)
    ba_ = np.asarray(ba, np.float64)
    gamma = np.asarray(gamma, np.float64)
    beta = np.asarray(beta, np.float64)
    Wout_ = np.asarray(Wout, np.float64)
    bout_ = np.asarray(bout, np.float64)

    Wcat = np.zeros((F_IN, HID))
    va1 = np.zeros((F_IN, HEADS))
    va2 = np.zeros((F_IN, HEADS))
    c1 = np.zeros(HEADS)
    c2 = np.zeros(HEADS)
    for h in range(HEADS):
        Wcat[:, h * HD : (h + 1) * HD] = W[h]
        va1[:, h] = W[h] @ A_[h, :HD]
        va2[:, h] = W[h] @ A_[h, HD:]
        c1[h] = bw_[h] @ A_[h, :HD] + ba_[h]
        c2[h] = bw_[h] @ A_[h, HD:]

    # ph columns: [0:256) H(no bias) | [256:272) sj dup | [272:288) si dup
    WCATA = np.zeros((2, 128, 288))
    BIASROW = np.zeros((1, 288))
    for k in range(2):
        WCATA[k, :, 0:256] = Wcat[k * 128 : (k + 1) * 128, :]
        for h in range(HEADS):
            for r in range(2):
                WCATA[k, :, 256 + 2 * h + r] = va2[k * 128 : (k + 1) * 128, h]
                WCATA[k, :, 272 + 2 * h + r] = va1[k * 128 : (k + 1) * 128, h]
    for h in range(HEADS):
        for r in range(2):
            BIASROW[0, 256 + 2 * h + r] = c2[h]
            BIASROW[0, 272 + 2 * h + r] = c1[h]

    WTILE = np.zeros((2, 128, HID))
    for f in range(F_IN):
        h, j = f // HD, f % HD
        WTILE[f // 128, f % 128, :] = gamma[h, j] * Wout_[j, :] / HEADS
    WEXTRA = np.zeros((9, HID))
    for h in range(HEADS):
        WEXTRA[h] = -(gamma[h] @ Wout_) / (HEADS * HD)
    WEXTRA[8] = bout_ + beta.mean(axis=0) @ Wout_

    padrow = np.zeros(TBW, ml_dtypes.bfloat16)
    padrow[256:272] = -240.0

    consts = dict(
        WCATA=WCATA.astype(ml_dtypes.bfloat16),
        BIASROW=BIASROW.astype(ml_dtypes.bfloat16),
        BWEXP=np.tile(bw_.reshape(1, -1), (P, 1)).astype(ml_dtypes.bfloat16),
        WTILE=WTILE.astype(ml_dtypes.bfloat16),
        WEXTRA=WEXTRA.astype(ml_dtypes.bfloat16),
        PADROW=padrow.reshape(1, TBW),
        ONESB=np.ones((1, 128), ml_dtypes.bfloat16),
        IDB=np.eye(P, dtype=ml_dtypes.bfloat16),
        IDF=np.eye(P, dtype=np.float32),
    )
    meta = dict(tile_cap=tile_cap, icols=icols, perm=perm)
    return meta, in_maps, consts


# ------------------------------------------------------------- device build
def _build(meta, consts):
    tile_cap, icols = meta["tile_cap"], meta["icols"]

    nc = bacc.Bacc(None, num_devices=NC)

    xT_d = nc.dram_tensor("xT", [2, 128, NPAD], BF16, kind="ExternalInput")
    idx_d = nc.dram_tensor("idx", [P, icols], I16, kind="ExternalInput")
    y_d = nc.dram_tensor("y", [LT * P, HID], BF16, kind="ExternalOutput")
    tbl_d = nc.dram_tensor("tbl", [NR, TBW], BF16)
    debug = os.environ.get("K2_DEBUG", "") == "1"
    if debug:
        dbg_tbl = nc.dram_tensor(
            "dbg_tbl", [NR, 2 * TBW], mybir.dt.uint8, kind="ExternalOutput"
        )

    cWCATA = nc.inline_tensor(np.asarray(consts["WCATA"]), "cWCATA")
    cBIASROW = nc.inline_tensor(np.asarray(consts["BIASROW"]), "cBIASROW")
    cBWEXP = nc.inline_tensor(np.asarray(consts["BWEXP"]), "cBWEXP")
    cWTILE = nc.inline_tensor(np.asarray(consts["WTILE"]), "cWTILE")
    cWEXTRA = nc.inline_tensor(np.asarray(consts["WEXTRA"]), "cWEXTRA")
    cPADROW = nc.inline_tensor(np.asarray(consts["PADROW"]), "cPADROW")
    cONESB = nc.inline_tensor(np.asarray(consts["ONESB"]), "cONESB")
    cIDB = nc.inline_tensor(np.asarray(consts["IDB"]), "cIDB")
    cIDF = nc.inline_tensor(np.asarray(consts["IDF"]), "cIDF")

    nidx_reg = {}
    for v in sorted({P * (int(c) + 1) for c in tile_cap}):
        r = nc.alloc_register(mybir.EngineType.Pool, f"nidx{v}")
        nc.gpsimd.reg_mov(r, v)
        nidx_reg[v] = r

    with tile.TileContext(nc) as tc:
        with tc.tile_pool(name="const", bufs=1) as cpool:
            WCATA = cpool.tile([128, 2, 288], BF16)
            BIASROW = cpool.tile([1, 288], BF16)
            BWEXP = cpool.tile([P, 256], BF16)
            WTILE = cpool.tile([128, 2, HID], BF16)
            WEXTRA = cpool.tile([9, HID], BF16)
            PADT = cpool.tile([1, TBW], BF16)
            ONESB = cpool.tile([1, 128], BF16)
            IDB = cpool.tile([P, P], BF16)
            IDF = cpool.tile([P, P], F32)
            EPSC = cpool.tile([P, 1], F32)
            nc.gpsimd.memset(EPSC[:], EPS)
            nc.sync.dma_start(WCATA[:], cWCATA[:].rearrange("k p n -> p k n"))
            nc.sync.dma_start(BIASROW[:], cBIASROW[:])
            nc.sync.dma_start(BWEXP[:], cBWEXP[:])
            nc.sync.dma_start(WTILE[:], cWTILE[:].rearrange("k p n -> p k n"))
            nc.sync.dma_start(WEXTRA[:], cWEXTRA[:])
            nc.sync.dma_start(PADT[:], cPADROW[:])
            nc.sync.dma_start(ONESB[:], cONESB[:])
            nc.sync.dma_start(IDB[:], cIDB[:])
            nc.sync.dma_start(IDF[:], cIDF[:])

            # ================= Phase A: full node table ====================
            with (
                tc.tile_pool(name="xp", bufs=2) as xpool,
                tc.tile_pool(name="ap", bufs=2) as apool,
                tc.tile_pool(name="psA", bufs=2, space="PSUM") as psA,
            ):
                for b in range(G_TILES // ABATCH):
                    n0 = b * ABATCH * P
                    xb = xpool.tile([128, 2, ABATCH * P], BF16, tag="xb")
                    nc.sync.dma_start(
                        xb[:],
                        xT_d[:, :, n0 : n0 + ABATCH * P].rearrange(
                            "k p n -> p k n"
                        ),
                    )
                    ph = psA.tile([P, ABATCH, 512], F32, tag="phA")
                    for k in range(ABATCH):
                        nc.tensor.matmul(
                            ph[:, k, 0:288], xb[:, 0, k * P : (k + 1) * P],
                            WCATA[:, 0, :], start=True, stop=False,
                        )
                        nc.tensor.matmul(
                            ph[:, k, 0:256], xb[:, 1, k * P : (k + 1) * P],
                            WCATA[:, 1, 0:256], start=False, stop=True,
                        )
                        nc.tensor.matmul(
                            ph[:, k, 256:288], xb[:, 1, k * P : (k + 1) * P],
                            WCATA[:, 1, 256:288], start=False, stop=False,
                        )
                        nc.tensor.matmul(
                            ph[:, k, 256:288], ONESB[:], BIASROW[:, 256:288],
                            start=False, stop=True,
                        )
                    t8 = apool.tile([P, ABATCH, 288], BF16, tag="t8")
                    nc.vector.tensor_tensor(
                        out=t8[:, :, 0:256],
                        in0=ph[:, :, 0:256],
                        in1=BWEXP[:].unsqueeze(1).to_broadcast(
                            [P, ABATCH, 256]
                        ),
                        op=OP.add,
                    )
                    nc.scalar.copy(t8[:, :, 256:288], ph[:, :, 256:288])
                    nc.sync.dma_start(
                        tbl_d[n0 : n0 + ABATCH * P, 0:288].rearrange(
                            "(k p) w -> p k w", p=P
                        ),
                        t8[:],
                    )
                nc.sync.dma_start(tbl_d[PADROW : PADROW + 1, :], PADT[:])
                if debug:
                    nc.sync.dma_start(
                        dbg_tbl[:], tbl_d[:].bitcast(mybir.dt.uint8)
                    )

            # ================= Phase B =====================================
            with (
                tc.tile_pool(name="sp", bufs=2) as spool,
                tc.tile_pool(name="gp", bufs=2) as gpool,
                tc.tile_pool(name="rp", bufs=2) as rpool,
                tc.tile_pool(name="pp", bufs=2) as ppool,
                tc.tile_pool(name="psB", bufs=2, space="PSUM") as psB,
                tc.tile_pool(name="psC", bufs=2, space="PSUM") as psC,
            ):
                n_pairs = (LT + 1) // 2
                icol = 0
                for pi in range(n_pairs):
                    tiles = [t for t in (2 * pi, 2 * pi + 1) if t < LT]
                    ntl = len(tiles)

                    hblk = spool.tile([P, 2, TBW], BF16, tag="hblk")
                    den = spool.tile([P, 2, HEADS], F32, tag="den")
                    pagg = psB.tile([P, 2, HID], F32, tag="ps_big")
                    for ti, t in enumerate(tiles):
                        cap = int(tile_cap[t]) + 1
                        rcap = cap - 1
                        nblk = (rcap + RB - 1) // RB
                        nc.sync.dma_start(
                            hblk[:, ti, :], tbl_d[t * P : (t + 1) * P, :]
                        )
                        idxt = spool.tile([P, 8 * cap], I16, tag="idxt")
                        nc.sync.dma_start(
                            idxt[:], idx_d[:, icol : icol + 8 * cap]
                        )
                        icol += 8 * cap

                        grid = gpool.tile([P, cap, TBW], BF16, tag="grid")
                        if NO_GATHER:
                            nc.gpsimd.memset(grid[:], 0)
                        else:
                            nc.gpsimd.dma_gather(
                                grid[:],
                                tbl_d[BASE:, :],
                                idxt[:],
                                P * cap,
                                nidx_reg[P * cap],
                                TBW,
                                single_packet=False,
                            )

                        egd = spool.tile([P, cap, 16], BF16, tag="egd")
                        nc.vector.tensor_tensor(
                            out=egd[:],
                            in0=grid[:, :, 256:272],
                            in1=hblk[:, ti, 272:288]
                            .unsqueeze(1)
                            .to_broadcast([P, cap, 16]),
                            op=OP.add,
                        )
                        nc.vector.scalar_tensor_tensor(
                            out=egd[:], in0=egd[:], scalar=SLOPE,
                            in1=egd[:], op0=OP.mult, op1=OP.max,
                        )
                        exd = spool.tile([P, cap, 16], BF16, tag="exd")
                        nc.scalar.activation(exd[:], egd[:], AF.Exp)
                        nc.vector.tensor_reduce(
                            den[:, ti, :],
                            exd[:]
                            .rearrange("p c (h two) -> p h two c", two=2)[
                                :, :, 0, :
                            ],
                            axis=AX.X,
                            op=OP.add,
                        )

                        # R = exp * H ; PE reduces slots (sentinel excluded)
                        ci = 0
                        for bb in range(nblk):
                            j0 = bb * RB
                            nb = min(RB, rcap - j0)
                            Rc = rpool.tile([P, RB, HID], BF16, tag="R")
                            nc.vector.tensor_tensor(
                                out=Rc[:, 0:nb, :].rearrange(
                                    "p c (h f two) -> p c h f two",
                                    h=HEADS, two=2,
                                ),
                                in0=grid[:, j0 : j0 + nb, 0:256].rearrange(
                                    "p c (h f two) -> p c h f two",
                                    h=HEADS, two=2,
                                ),
                                in1=exd[:, j0 : j0 + nb, :]
                                .rearrange("p c (h two) -> p c h two", two=2)
                                .unsqueeze(3)
                                .to_broadcast([P, nb, HEADS, HD // 2, 2]),
                                op=OP.mult,
                            )
                            for j in range(nb):
                                nc.tensor.matmul(
                                    pagg[:, ti, :],
                                    IDB[:],
                                    Rc[:, j, :],
                                    start=(ci == 0),
                                    stop=(ci == rcap - 1),
                                )
                                ci += 1

                    nc.vector.tensor_scalar_max(den[:], den[:], 1e-30)
                    rden = spool.tile([P, 2, HEADS], F32, tag="rden")
                    nc.vector.reciprocal(rden[:], den[:])

                    # ---- post (per pair): normalize, skip, ELU, LN, out, ELU
                    ob = ppool.tile([P, 2, HID], BF16, tag="ob")
                    nc.vector.tensor_tensor(
                        out=ob[:, 0:ntl, :].rearrange(
                            "p t (h f) -> p t h f", h=HEADS
                        ),
                        in0=pagg[:, 0:ntl, :].rearrange(
                            "p t (h f) -> p t h f", h=HEADS
                        ),
                        in1=rden[:, 0:ntl, :]
                        .unsqueeze(3)
                        .to_broadcast([P, ntl, HEADS, HD]),
                        op=OP.mult,
                    )
                    nc.vector.tensor_tensor(
                        out=ob[:, 0:ntl, :],
                        in0=ob[:, 0:ntl, :],
                        in1=hblk[:, 0:ntl, 0:256],
                        op=OP.add,
                    )
                    t1 = ppool.tile([P, 2, HID], BF16, tag="t1")
                    nc.scalar.activation(
                        t1[:, 0:ntl, :], ob[:, 0:ntl, :], AF.Relu, scale=-1.0
                    )
                    nc.scalar.activation(
                        t1[:, 0:ntl, :], t1[:, 0:ntl, :], AF.Exp, scale=-1.0
                    )
                    elu = ppool.tile([P, 2, HID], BF16, tag="elu")
                    nc.vector.scalar_tensor_tensor(
                        out=elu[:, 0:ntl, :],
                        in0=t1[:, 0:ntl, :],
                        scalar=-1.0,
                        in1=ob[:, 0:ntl, :],
                        op0=OP.add,
                        op1=OP.max,
                    )

                    nh = ntl * HEADS
                    st = ppool.tile([P, 8, 2 * HEADS], F32, tag="st")
                    r1, r2, mu2, var, sd, rr, tmp, _ = (
                        st[:, i, :] for i in range(8)
                    )
                    nc.vector.tensor_reduce(
                        r1[:, 0:nh],
                        elu[:, 0:ntl, :].rearrange(
                            "p t (h f) -> p (t h) f", f=HD
                        ),
                        axis=AX.X,
                        op=OP.add,
                    )
                    sq = ppool.tile([P, 2, HID], BF16, tag="t1")
                    nc.scalar.activation(
                        sq[:, 0:ntl, :], elu[:, 0:ntl, :], AF.Square
                    )
                    nc.vector.tensor_reduce(
                        r2[:, 0:nh],
                        sq[:, 0:ntl, :].rearrange(
                            "p t (h f) -> p (t h) f", f=HD
                        ),
                        axis=AX.X,
                        op=OP.add,
                    )
                    nc.scalar.activation(
                        mu2[:, 0:nh], r1[:, 0:nh], AF.Square, scale=1.0 / HD
                    )
                    nc.vector.scalar_tensor_tensor(
                        out=var[:, 0:nh], in0=r2[:, 0:nh], scalar=1.0 / HD,
                        in1=mu2[:, 0:nh], op0=OP.mult, op1=OP.subtract,
                    )
                    nc.scalar.activation(
                        sd[:, 0:nh], var[:, 0:nh], AF.Sqrt, bias=EPSC[:]
                    )
                    nc.vector.reciprocal(rr[:, 0:nh], sd[:, 0:nh])
                    nc.vector.tensor_tensor(
                        out=tmp[:, 0:nh], in0=r1[:, 0:nh], in1=rr[:, 0:nh],
                        op=OP.mult,
                    )

                    xw = ppool.tile([P, 2, HID], BF16, tag="xw")
                    nc.vector.tensor_tensor(
                        out=xw[:, 0:ntl, :].rearrange(
                            "p t (h f) -> p t h f", h=HEADS
                        ),
                        in0=elu[:, 0:ntl, :].rearrange(
                            "p t (h f) -> p t h f", h=HEADS
                        ),
                        in1=rr[:, 0:nh]
                        .rearrange("p (t h) -> p t h", h=HEADS)
                        .unsqueeze(3)
                        .to_broadcast([P, ntl, HEADS, HD]),
                        op=OP.mult,
                    )

                    py = psB.tile([P, 2, HID], F32, tag="ps_big")
                    yb = ppool.tile([P, 2, HID], BF16, tag="yb")
                    for ti, t in enumerate(tiles):
                        xwT = spool.tile([P, 2, P], BF16, tag="xwT")
                        for k in range(2):
                            pt = psC.tile([P, P], BF16, tag="ps_tr")
                            nc.tensor.transpose(
                                pt[:], xw[:, ti, k * P : (k + 1) * P], IDB[:]
                            )
                            nc.scalar.copy(xwT[:, k, :], pt[:])
                        t9 = spool.tile([P, 9], F32, tag="t9")
                        nc.scalar.copy(
                            t9[:, 0:8], tmp[:, ti * HEADS : (ti + 1) * HEADS]
                        )
                        nc.scalar.activation(
                            t9[:, 8:9], t9[:, 0:1], AF.Copy,
                            scale=0.0, bias=1.0,
                        )
                        ptm = psC.tile([P, P], F32, tag="ps_trf")
                        nc.tensor.transpose(ptm[0:9, :], t9[:], IDF[:])
                        exT = spool.tile([9, P], BF16, tag="exT")
                        nc.scalar.copy(exT[:], ptm[0:9, :])

                        nc.tensor.matmul(
                            py[:, ti, :], xwT[:, 0, :], WTILE[:, 0, :],
                            start=True, stop=False,
                        )
                        nc.tensor.matmul(
                            py[:, ti, :], xwT[:, 1, :], WTILE[:, 1, :],
                            start=False, stop=False,
                        )
                        nc.tensor.matmul(
                            py[:, ti, :], exT[:], WEXTRA[:],
                            start=False, stop=True,
                        )
                    nc.scalar.copy(yb[:, 0:ntl, :], py[:, 0:ntl, :])
                    t2 = ppool.tile([P, 2, HID], BF16, tag="t2")
                    nc.scalar.activation(
                        t2[:, 0:ntl, :], yb[:, 0:ntl, :], AF.Relu, scale=-1.0
                    )
                    nc.scalar.activation(
                        t2[:, 0:ntl, :], t2[:, 0:ntl, :], AF.Exp, scale=-1.0
                    )
                    ysb = ppool.tile([P, 2, HID], BF16, tag="ysb")
                    nc.vector.scalar_tensor_tensor(
                        out=ysb[:, 0:ntl, :],
                        in0=t2[:, 0:ntl, :],
                        scalar=-1.0,
                        in1=yb[:, 0:ntl, :],
                        op0=OP.add,
                        op1=OP.max,
                    )
                    for ti, t in enumerate(tiles):
                        nc.sync.dma_start(
                            y_d[t * P : (t + 1) * P, :], ysb[:, ti, :]
                        )

    nc.compile()
    return nc


# ------------------------------------------------------------------ driver
_CACHE = {}


def kernel(**inputs):
    meta, in_maps, consts = _prepare(**inputs)
    key = tuple(meta["tile_cap"].tolist())
    if key not in _CACHE:
        _CACHE[key] = _build(meta, consts)
    nc = _CACHE[key]

    from concourse.bass_utils import run_bass_kernel_spmd

    global LAST_NC, LAST_INMAPS
    LAST_NC = nc
    LAST_INMAPS = in_maps

    res = run_bass_kernel_spmd(nc, in_maps, core_ids=list(range(NC)))
    global LAST_RESULT
    LAST_RESULT = res
    outs = res.results

    y_all = np.zeros((NPAD, HID), np.float32)
    for c in range(NC):
        g_idx = (np.arange(LT) * NC + c)[:, None] * P + np.arange(P)[None, :]
        y_all[g_idx.reshape(-1)] = outs[c]["y"].astype(np.float32)
    y = np.zeros((N_NODES, HID), np.float32)
    y[meta["perm"]] = y_all[:N_NODES]
    return y
